# revision 1
# baseline (speedup 1.0000x reference)
"""Trainium2 Bass kernel for nn_AMIPRouterTrain (moe_routing).

Computes, for full inputs (N=4096 tokens, D=4096):
    weights = softmax(h_mask @ Wr + br)                      # [N, 8]
    cond    = concat([h_anchor, h_mask], -1)                 # [N, 8192]
    hid     = gelu(einsum('nd,kdf->knf', cond, W1) + b1)     # [8, N, 2048]
    eout    = einsum('knf,kfd->knd', hid, W2) + b2           # [8, N, 4096]
    delta   = einsum('knd,nk->nd', eout, weights)
    rel     = sigmoid(sum((h_mask@Wq+bq)*(h_anchor@Wk+bk), -1, keep) / sqrt(512))
    out     = delta * rel

Distribution over 8 NeuronCores (one trn2 chip):
  - Expert-parallel: core k owns expert k (W1[k]/W2[k] streamed from HBM),
    computes w_k-weighted expert output for ALL tokens, in bf16 with f32 PSUM
    accumulation.  The relevance gate is folded into the per-token scale
    (w_k * rel), so the cross-core combine is a plain sum (ReduceScatter
    per 512-token block; core i receives rows [b*512+64i, b*512+64(i+1))).
  - The conditioned activations are fully replicated in each core's HBM
    (packed host-side, p-major), so no AllGather and no transpose-shaped
    DMA descriptors; each 512-token block is one contiguous 8.4MB load.
  - Router logits / q / k are computed in column layout (dims on
    partitions, tokens on the free axis) with the small weight matrices as
    the matmul stationary: 64+2 wide matmuls per block instead of ~264
    token-tile-stationary ones.  Softmax over the 8 logits runs across
    partitions (exp on ACT, partition-sum via a ones-vector matmul); the
    q.k dot is sharded over the 512 projection dims (64 per core) and
    combined with a tiny [1,512] AllReduce per block, issued right after
    the router so its latency hides behind stage-1 (~200us).
  - The per-token scale (w_mine * rel) is transposed from a [1,512] row
    into [128,4] columns with 4 tiny PE transposes and applied to the
    stage-2 PSUM tiles during eviction.
"""

import os
import sys

for _p in ("/opt/trn_rl_repo", "/root/.axon_site/_ro/trn_rl_repo"):
    if os.path.isdir(_p) and _p not in sys.path:
        sys.path.insert(0, _p)

import numpy as np
import ml_dtypes

BF16 = ml_dtypes.bfloat16

# Problem dims (hardcoded per spec)
D = 4096          # d_model
NEXP = 8          # experts
DH = 2048         # expert hidden
DP = 512          # gate projection dim
NTOK = 4096       # tokens

N_CORES = 8
TB = 512              # tokens per block
NB = NTOK // TB       # 8 blocks
TT = TB // 128        # 4 token tiles per block
FT = DH // 128        # 16 f-tiles (stage-1 outputs / stage-2 contraction)
DC = (2 * D) // 128   # 64 contraction chunks for stage 1
DT = D // 512         # 8 d-tiles for stage 2
PS = DP // N_CORES    # 64 projection dims per core
RW = 128              # router+gate column width (8 logits, pad, 64 q dims)
QOF = 64              # q/k rows live at partitions 64..127 (tile_position
                      # requires 64-row matmul operands at partition 0/64)
RSQRT_DP = float(1.0 / np.sqrt(np.float32(DP)))

_PATCHED = False
_RUNNERS = {}


def _patch_drain():
    """This neuronxcc rejects instructions with >1-2 sem waits on the
    sequencer-only Drain at TileContext exit.  Split the waits across
    chained NOPs (sync engine, program order) — semantics preserved since
    every wait still precedes the final barrier."""
    global _PATCHED
    if _PATCHED:
        return
    import concourse.mybir as mybir
    import concourse.tile as tile
    from concourse.vector_clock import ScopedClock

    def _drain_and_barrier(self, tick_clock, wait_clock):
        drain_inst = self.nc.sync.drain()
        wait_clock.add_sem_waits(
            drain_inst.ins, ScopedClock({None: tick_clock.global_clock})
        )
        ins = drain_inst.ins
        si = ins.sync_info
        waits = list(si.on_wait)
        if len(waits) > 1:
            ins.sync_info = mybir.SyncInfo(
                on_wait=[waits[0]], on_update=list(si.on_update)
            )
            for w in waits[1:]:
                nop = self.nc.sync.nop(nofuse=True, hint="drain_wait_split")
                nop.ins.sync_info = mybir.SyncInfo(on_wait=[w], on_update=[])
        self.nc.all_engine_barrier()
        assert self.sems is not None
        popped = self.nc._tile_sem_poison_stack.pop()
        assert popped is self._sem_poison
        self.nc.clear_and_free_semaphores(list(self.sems.allocated().values()))
        self.nc.all_engine_barrier()

    tile.TileContext._drain_and_barrier = _drain_and_barrier

    # Redundant-LDWEIGHTS elision: stage 2 issues runs of 4 matmuls sharing
    # one stationary; concourse legalization still emits one Ldweights per
    # Matmult.  Walrus's own ldw-opt rejects pre-split Ldweights, so strip
    # the redundant reloads from the BIR JSON on its way to walrus: an
    # Ldweights with no sem waits/updates whose weights AP equals the PE
    # array's current contents is a no-op.
    import json as _json

    from concourse import bass2jax as _b2j
    from concourse import bass_utils as _bu

    _orig_cbk = _bu.compile_bir_kernel

    def _elide_ldw(bir_json, *args, **kwargs):
        bir = _json.loads(bir_json)
        removed = 0
        for fn in bir.get("functions", []):
            for blk in fn.get("blocks", []):
                il = blk.get("instructions")
                if not il:
                    continue
                last = None
                keep = []
                for ins in il:
                    if ins.get("engine") == "PE":
                        op = ins.get("opcode")
                        if op == "Ldweights":
                            si = ins.get("sync_info") or {}
                            sig = _json.dumps(
                                [ins.get("ins"), ins.get("tile_position"),
                                 ins.get("tile_size"), ins.get("perf_mode"),
                                 ins.get("is_transpose")],
                                sort_keys=True,
                            )
                            if (sig == last and not si.get("on_wait")
                                    and not si.get("on_update")):
                                removed += 1
                                continue
                            last = sig
                        elif op != "Matmult":
                            last = None
                    keep.append(ins)
                blk["instructions"] = keep
        if removed:
            print(f"[kernel] elided {removed} redundant Ldweights")
        return _orig_cbk(_json.dumps(bir).encode(), *args, **kwargs)

    _bu.compile_bir_kernel = _elide_ldw
    _b2j.compile_bir_kernel = _elide_ldw
    _PATCHED = True


def build_graph(repeat: int = 1):
    """Build the SPMD Bass graph (same on all 8 cores)."""
    _patch_drain()
    import concourse.bacc as bacc
    import concourse.mybir as mybir
    import concourse.tile as tile

    f32 = mybir.dt.float32
    bf = mybir.dt.bfloat16
    AF = mybir.ActivationFunctionType
    RG = [list(range(N_CORES))]

    nc = bacc.Bacc(num_devices=N_CORES)

    # All params are packed host-side so every DMA below is p-major
    # contiguous (or >=1KB runs).
    SH = TB // N_CORES  # 64 tokens contributed per core per block
    xsh = nc.declare_dram_parameter("xsh", [NB, 128, DC, SH], bf, isOutput=False)
    w1 = nc.declare_dram_parameter("w1", [FT, 128, DC, 128], bf, isOutput=False)
    w2 = nc.declare_dram_parameter("w2", [128, FT, D], bf, isOutput=False)
    wrq = nc.declare_dram_parameter("wrq", [128, 32, RW], bf, isOutput=False)
    wkp = nc.declare_dram_parameter("wk", [128, 32, RW], bf, isOutput=False)
    b1p = nc.declare_dram_parameter("b1", [128, FT], f32, isOutput=False)
    b2p = nc.declare_dram_parameter("b2", [1, D], bf, isOutput=False)
    brq = nc.declare_dram_parameter("brq", [1, RW], bf, isOutput=False)
    bkp = nc.declare_dram_parameter("bk", [1, RW], bf, isOutput=False)
    out = nc.declare_dram_parameter("out", [NTOK // N_CORES, D], bf, isOutput=True)

    with tile.TileContext(nc) as tc:
        with tc.tile_pool(name="res", bufs=1) as res, \
             tc.tile_pool(name="xp", bufs=1) as xp, \
             tc.tile_pool(name="w1p", bufs=2) as w1p, \
             tc.tile_pool(name="hidp", bufs=2 * FT) as hidp, \
             tc.tile_pool(name="w2p", bufs=2) as w2p, \
             tc.tile_pool(name="outp", bufs=2) as outp, \
             tc.tile_pool(name="sm", bufs=1) as sm, \
             tc.tile_pool(name="pspool", bufs=8, space="PSUM") as pspool, \
             tc.tile_pool(name="dramp", bufs=2, space="DRAM") as dramp:

            # --- resident small tensors ---
            wrq_sb = res.tile([128, 32, RW], bf, name="wrq_sb")
            nc.scalar.dma_start(wrq_sb[:], wrq.ap())
            wk_sb = res.tile([128, 32, RW], bf, name="wk_sb")
            nc.scalar.dma_start(wk_sb[:], wkp.ap())
            b1_sb = res.tile([128, FT], f32, name="b1_sb")
            nc.scalar.dma_start(b1_sb[:], b1p.ap())
            b2_sb = res.tile([1, D], bf, name="b2_sb")
            nc.scalar.dma_start(b2_sb[:], b2p.ap())
            brq_sb = res.tile([1, RW], bf, name="brq_sb")
            nc.scalar.dma_start(brq_sb[:], brq.ap())
            bk_sb = res.tile([1, RW], bf, name="bk_sb")
            nc.scalar.dma_start(bk_sb[:], bkp.ap())
            ones_row = res.tile([1, TB], bf, name="ones_row")
            nc.vector.memset(ones_row[:], 1.0)
            ones_col = res.tile([128, 1], bf, name="ones_col")
            nc.vector.memset(ones_col[:], 1.0)
            one1 = res.tile([1, 1], bf, name="one1")
            nc.vector.memset(one1[:], 1.0)

            def issue_ag(b, split=False):
                # gather block b's activations from all cores (core i
                # contributes tokens [b*512+64i, b*512+64(i+1))); with
                # split=True in (mask, anchor) halves so block 0's router
                # can start after only the mask half has arrived
                parts = []
                ranges = [(32, DC), (0, 32)] if split else [(0, DC)]
                for lo, hi in ranges:
                    w = hi - lo
                    ag_in = dramp.tile([128, w, SH], bf, name="ag_in",
                                       tag="ag_in")
                    nc.gpsimd.dma_start(ag_in[:], xsh[b][:, lo:hi, :])
                    ag_out = dramp.tile([N_CORES, 128, w, SH], bf,
                                        name="ag_out", tag="ag_out",
                                        addr_space="Shared")
                    nc.gpsimd.collective_compute(
                        "AllGather", mybir.AluOpType.bypass,
                        replica_groups=RG,
                        ins=[ag_in.opt()], outs=[ag_out.opt()],
                    )
                    parts.append((ag_out, lo, hi))
                return parts

            ag_next = issue_ag(0, split=True)
            steps = [(r, b) for r in range(repeat) for b in range(NB)]
            for _si, (_r, b) in enumerate(steps):
                # ---- merge this block's AllGathered activations ----------
                # On sync, ahead of the block's w1t stream: transfers during
                # the previous block's stage-2.  Both sides p-major
                # contiguous (4-8KB runs per partition).
                xb = xp.tile([128, N_CORES, DC, SH], bf, name="xb", tag="xb")
                for ag_out, lo, hi in ag_next:
                    for i in range(N_CORES):
                        nc.sync.dma_start(xb[:, i, lo:hi, :], ag_out[i])

                def xch(c):
                    # [128, (8, 64)] chunk c of the block, tokens in global
                    # (core, slot) order along the (multi-dim) free axis
                    return xb[:, :, c, :]

                # ---- router + gate, column layout -----------------------
                # prq rows 0..7 = my-expert-rolled logits, rows 8..71 = q
                prq = pspool.tile([RW, TB], f32, name="prq", tag="ps")
                for c in range(32):
                    nc.tensor.matmul(
                        prq[:], lhsT=wrq_sb[:, c, :], rhs=xch(32 + c),
                        start=(c == 0), stop=False,
                    )
                nc.tensor.matmul(
                    prq[:], lhsT=brq_sb[:], rhs=ones_row[:],
                    start=False, stop=True,
                )
                # pk rows 8..71 = k (cols 0..7 of wk are zero-padded)
                pk = pspool.tile([RW, TB], f32, name="pk", tag="ps")
                for c in range(32):
                    nc.tensor.matmul(
                        pk[:], lhsT=wk_sb[:, c, :], rhs=xch(c),
                        start=(c == 0), stop=False,
                    )
                nc.tensor.matmul(
                    pk[:], lhsT=bk_sb[:], rhs=ones_row[:],
                    start=False, stop=True,
                )
                # softmax numerator (no max-shift: logits ~ N(0,1))
                ex = sm.tile([8, TB], f32, name="ex", tag="ex")
                nc.scalar.activation(ex[:], prq[0:8, :], AF.Exp)
                exb = sm.tile([8, TB], bf, name="exb", tag="exb")
                nc.vector.tensor_copy(exb[:], ex[:])
                # q*k partial for my 64 projection dims (DVE reads one PSUM
                # operand at most: stage q rows through SBUF on ACT first)
                qsb = sm.tile([128, TB], bf, name="qsb", tag="qsb")
                nc.scalar.copy(qsb[QOF:RW, :], prq[QOF:RW, :])
                qk = sm.tile([128, TB], bf, name="qk", tag="qk")
                nc.vector.tensor_mul(qk[QOF:RW, :], qsb[QOF:RW, :], pk[QOF:RW, :])
                psg = pspool.tile([1, TB], f32, name="psg", tag="ps")
                nc.tensor.matmul(
                    psg[:], lhsT=ones_col[QOF:RW, :], rhs=qk[QOF:RW, :],
                    start=True, stop=True,
                )
                qks = sm.tile([1, TB], f32, name="qks", tag="qks")
                nc.vector.tensor_copy(qks[:], psg[:])
                # softmax denominator
                pss = pspool.tile([1, TB], f32, name="pss", tag="ps")
                nc.tensor.matmul(
                    pss[:], lhsT=ones_col[0:8, :], rhs=exb[:],
                    start=True, stop=True,
                )
                rcp = sm.tile([1, TB], f32, name="rcp", tag="rcp")
                nc.vector.reciprocal(rcp[:], pss[:])
                wmine = sm.tile([1, TB], f32, name="wmine", tag="wmine")
                nc.vector.tensor_mul(wmine[:], ex[0:1, :], rcp[:])

                # tiny AllReduce of the gate partials for this block
                ar_in = dramp.tile([1, TB], f32, name="ar_in", tag="ar_in")
                nc.gpsimd.dma_start(ar_in[:], qks[:])
                ar_out = dramp.tile([1, TB], f32, name="ar_out",
                                    tag="ar_out", addr_space="Shared")
                nc.gpsimd.collective_compute(
                    "AllReduce", mybir.AluOpType.add, replica_groups=RG,
                    ins=[ar_in.opt()], outs=[ar_out.opt()],
                )
                # prefetch next block's activations behind the AllReduce
                if _si + 1 < len(steps):
                    ag_next = issue_ag(steps[_si + 1][1])

                # ---- stage 1: hidT[ft] = gelu(W1^T cond^T + b1) ---------
                hid = []
                for ft in range(FT):
                    w1t = w1p.tile([128, DC, 128], bf, name="w1t", tag="w1t")
                    nc.sync.dma_start(w1t[:], w1[ft])
                    ps1 = pspool.tile([128, TB], f32, name="ps1", tag="ps")
                    # mask chunks first: on block 0 the anchor half of the
                    # split AllGather lands second
                    for ci, c in enumerate(
                        list(range(32, DC)) + list(range(32))
                    ):
                        nc.tensor.matmul(
                            ps1[:], lhsT=w1t[:, c, :], rhs=xch(c),
                            start=(ci == 0), stop=(ci == DC - 1),
                        )
                    ht = hidp.tile([128, TB], bf, name="hid", tag="hid")
                    nc.scalar.activation(
                        ht[:], ps1[:], AF.Gelu, bias=b1_sb[:, ft:ft + 1]
                    )
                    hid.append(ht)

                # ---- finish the gate: rel, then w_mine*rel as columns ---
                qkt = sm.tile([1, TB], f32, name="qkt", tag="qkt")
                nc.gpsimd.dma_start(qkt[:], ar_out[:])
                rel = sm.tile([1, TB], f32, name="rel", tag="rel")
                nc.scalar.activation(rel[:], qkt[:], AF.Sigmoid, scale=RSQRT_DP)
                wrow = sm.tile([1, TB], bf, name="wrow", tag="wrow")
                nc.vector.tensor_mul(wrow[:], wmine[:], rel[:])
                # row->column via K=1 matmuls: out[p,0] = wrow[0, t*128+p]
                psw = pspool.tile([128, TT], f32, name="psw", tag="ps")
                for t in range(TT):
                    nc.tensor.matmul(
                        psw[:, t:t + 1],
                        lhsT=wrow[:, t * 128:(t + 1) * 128], rhs=one1[:],
                        start=True, stop=True,
                    )
                wrel = sm.tile([128, TT], f32, name="wrel", tag="wrel",
                               bufs=2)
                nc.vector.tensor_copy(wrel[:], psw[:])

                # ---- stage 2: delta = (w*rel) * (hidT^T @ W2 + b2) ------
                # d-axis in two halves, each ReduceScattered separately so
                # the last block's collective tail is halved; token axis in
                # pairs of 128-tiles so the 8 PSUM banks cover (2 tok x 4 d)
                # and each hid stationary serves 4 consecutive matmuls
                # (redundant LDWEIGHTS elided by walrus ldw-opt).
                DG = D // 2
                for dtig in range(2):
                    dgs = slice(dtig * DG, (dtig + 1) * DG)
                    rs_in = dramp.tile([TB, DG], bf, name="rs_in", tag="rs_in")
                    for tp in range(2):
                        pst = [
                            pspool.tile([128, 512], f32, name="ps2", tag="ps")
                            for _ in range(8)
                        ]  # pst[tloc*4 + dj]
                        for fh in range(8):
                            w2h = w2p.tile([128, 2, DG], bf, name="w2h",
                                           tag="w2h")
                            nc.scalar.dma_start(
                                w2h[:], w2[:, fh * 2:(fh + 1) * 2, dgs]
                            )
                            for f2i in range(2):
                                f2 = fh * 2 + f2i
                                for tloc in range(2):
                                    t = 2 * tp + tloc
                                    lt = hid[f2][:, t * 128:(t + 1) * 128]
                                    for dj in range(4):
                                        nc.tensor.matmul(
                                            pst[tloc * 4 + dj][:],
                                            lhsT=lt,
                                            rhs=w2h[:, f2i,
                                                    dj * 512:(dj + 1) * 512],
                                            start=(f2 == 0), stop=False,
                                        )
                        ob = outp.tile([128, 2, DG], bf, name="ob", tag="ob")
                        for tloc in range(2):
                            t = 2 * tp + tloc
                            for dj in range(4):
                                dsl = slice(dtig * DG + dj * 512,
                                            dtig * DG + (dj + 1) * 512)
                                nc.tensor.matmul(
                                    pst[tloc * 4 + dj][:],
                                    lhsT=ones_row[:, 0:128],
                                    rhs=b2_sb[:, dsl], start=False, stop=True,
                                )
                                nc.vector.tensor_scalar_mul(
                                    ob[:, tloc, dj * 512:(dj + 1) * 512],
                                    pst[tloc * 4 + dj][:], wrel[:, t:t + 1],
                                )
                        nc.scalar.dma_start(
                            rs_in[tp * 256:(tp + 1) * 256, :]
                            .rearrange("(t p) d -> p t d", p=128),
                            ob[:],
                        )
                    # combine the 8 experts' weighted deltas for this d-half;
                    # core i receives rows [64*i, 64*(i+1)) of this block
                    rs_out = dramp.tile([TB // N_CORES, DG], bf,
                                        name="rs_out", tag="rs_out")
                    nc.gpsimd.collective_compute(
                        "ReduceScatter", mybir.AluOpType.add,
                        replica_groups=RG,
                        ins=[rs_in.opt()], outs=[rs_out.opt()],
                    )
                    ob_sl = slice(b * (TB // N_CORES),
                                  (b + 1) * (TB // N_CORES))
                    nc.gpsimd.dma_start(out.ap()[ob_sl, dgs], rs_out[:])

    nc.compile()
    return nc


class _Runner:
    def __init__(self, repeat: int):
        import jax
        from jax.sharding import Mesh, PartitionSpec
        from jax.experimental.shard_map import shard_map
        import concourse.mybir as mybir
        from concourse import bass2jax

        bass2jax.install_neuronx_cc_hook()
        nc = build_graph(repeat)
        self.nc = nc
        partition_name = (
            nc.partition_id_tensor.name if nc.partition_id_tensor else None
        )
        in_names, out_names, out_avals, zero_outs = [], [], [], []
        for alloc in nc.m.functions[0].allocations:
            if not isinstance(alloc, mybir.MemoryLocationSet):
                continue
            name = alloc.memorylocations[0].name
            if alloc.kind == "ExternalInput":
                if name != partition_name:
                    in_names.append(name)
            elif alloc.kind == "ExternalOutput":
                shape = tuple(alloc.tensor_shape)
                dtype = mybir.dt.np(alloc.dtype)
                out_names.append(name)
                out_avals.append(jax.core.ShapedArray(shape, dtype))
                zero_outs.append(np.zeros(shape, dtype))
        self.in_names = in_names
        self.out_names = out_names
        self.out_avals = out_avals
        self.zero_outs = zero_outs
        n_params = len(in_names)
        n_outs = len(out_avals)
        all_in = list(in_names) + list(out_names)
        if partition_name is not None:
            all_in.append(partition_name)

        def _body(*args):
            operands = list(args)
            if partition_name is not None:
                operands.append(bass2jax.partition_id_tensor())
            outs = bass2jax._bass_exec_p.bind(
                *operands,
                out_avals=tuple(out_avals),
                in_names=tuple(all_in),
                out_names=tuple(out_names),
                lowering_input_output_aliases=(),
                sim_require_finite=True,
                sim_require_nnan=True,
                nc=nc,
            )
            return tuple(outs)

        devices = jax.devices()[:N_CORES]
        assert len(devices) == N_CORES, f"need {N_CORES} cores, got {len(devices)}"
        mesh = Mesh(np.asarray(devices), ("core",))
        in_specs = (PartitionSpec("core"),) * (n_params + n_outs)
        out_specs = (PartitionSpec("core"),) * n_outs
        self.fn = jax.jit(
            shard_map(_body, mesh=mesh, in_specs=in_specs, out_specs=out_specs,
                      check_rep=False),
            keep_unused=True,
        )
        self._dev_zeros = None

    def dev_zeros(self):
        import jax
        if self._dev_zeros is None:
            self._dev_zeros = [
                jax.device_put(
                    np.zeros((N_CORES * z.shape[0], *z.shape[1:]), z.dtype)
                )
                for z in self.zero_outs
            ]
        return self._dev_zeros


def _get_runner(repeat: int = 1) -> "_Runner":
    if repeat not in _RUNNERS:
        _RUNNERS[repeat] = _Runner(repeat)
    return _RUNNERS[repeat]


def pack_inputs(inputs: dict) -> dict:
    """Per-core in_maps, concatenated along axis 0 (shard_map layout)."""
    h_anchor = np.asarray(inputs["h_anchor"], np.float32)
    h_mask = np.asarray(inputs["h_mask"], np.float32)
    Wr = np.asarray(inputs["Wr"], np.float32)
    br = np.asarray(inputs["br"], np.float32)
    W1 = np.asarray(inputs["W1"], np.float32)
    b1 = np.asarray(inputs["b1"], np.float32)
    W2 = np.asarray(inputs["W2"], np.float32)
    b2 = np.asarray(inputs["b2"], np.float32)
    Wq = np.asarray(inputs["Wq"], np.float32)
    bq = np.asarray(inputs["bq"], np.float32)
    Wk = np.asarray(inputs["Wk"], np.float32)
    bk = np.asarray(inputs["bk"], np.float32)

    xT = np.ascontiguousarray(
        np.concatenate([h_anchor.T, h_mask.T], axis=0)
    ).astype(BF16)                                             # [8192, 4096]
    SH = TB // N_CORES
    # [d-chunk, d-in-chunk, block, core, slot]
    xTr = xT.reshape(DC, 128, NB, N_CORES, SH)

    per = {k: [] for k in
           ("xsh", "w1", "w2", "wrq", "wk", "b1", "b2", "brq", "bk")}
    for k in range(N_CORES):
        psl = slice(PS * k, PS * (k + 1))
        per["xsh"].append(
            np.ascontiguousarray(xTr[:, :, :, k, :].transpose(2, 1, 0, 3))
        )                                                      # [NB,128,DC,SH]
        per["w1"].append(
            np.ascontiguousarray(
                W1[k].reshape(DC, 128, FT, 128).transpose(2, 1, 0, 3)
            ).astype(BF16)
        )                                                      # [FT,128,DC,128]
        per["w2"].append(
            np.ascontiguousarray(
                W2[k].reshape(FT, 128, D).transpose(1, 0, 2)
            ).astype(BF16)
        )                                                      # [128,FT,D]
        wr_k = np.roll(Wr, -k, axis=1)
        wrq_pad = np.concatenate(
            [wr_k, np.zeros((D, QOF - 8), np.float32), Wq[:, psl]], axis=1
        )
        per["wrq"].append(
            np.ascontiguousarray(
                wrq_pad.reshape(32, 128, RW).transpose(1, 0, 2)
            ).astype(BF16)
        )                                                      # [128,32,RW]
        wk_pad = np.concatenate(
            [np.zeros((D, QOF), np.float32), Wk[:, psl]], axis=1
        )
        per["wk"].append(
            np.ascontiguousarray(
                wk_pad.reshape(32, 128, RW).transpose(1, 0, 2)
            ).astype(BF16)
        )                                                      # [128,32,RW]
        per["b1"].append(
            np.ascontiguousarray(b1[k].reshape(FT, 128).T).astype(np.float32)
        )                                                      # [128,FT]
        per["b2"].append(b2[k][None].astype(BF16))
        per["brq"].append(
            np.concatenate(
                [np.roll(br, -k), np.zeros(QOF - 8, np.float32), bq[psl]]
            )[None].astype(BF16)
        )
        per["bk"].append(
            np.concatenate(
                [np.zeros(QOF, np.float32), bk[psl]]
            )[None].astype(BF16)
        )
    return {k: np.concatenate(v, axis=0) for k, v in per.items()}


def unshard_output(out_concat: np.ndarray) -> np.ndarray:
    """[8*512, 4096] concat of per-core shards -> full [4096, 4096] f32.

    Core i's shard rows are ordered block-major: row b*64+j of core i is
    global token b*512 + i*64 + j."""
    per = out_concat.astype(np.float32).reshape(N_CORES, NB, TB // N_CORES, D)
    return np.ascontiguousarray(
        per.transpose(1, 0, 2, 3).reshape(NTOK, D)
    )


def kernel(**inputs) -> np.ndarray:
    import jax

    runner = _get_runner(repeat=1)
    arrs = pack_inputs(inputs)
    dev = [jax.device_put(arrs[n]) for n in runner.in_names]
    outs = runner.fn(*dev, *runner.dev_zeros())
    out_concat = np.asarray(outs[0])
    return unshard_output(out_concat).astype(np.float32)


if __name__ == "__main__":
    # tiny self-driven sanity run with random data
    rng = np.random.default_rng(0)
    inputs = {
        "h_anchor": rng.standard_normal((NTOK, D)).astype(np.float32),
        "h_mask": rng.standard_normal((NTOK, D)).astype(np.float32),
        "Wr": (rng.standard_normal((D, NEXP)) / np.sqrt(D)).astype(np.float32),
        "br": np.zeros(NEXP, np.float32),
        "W1": (rng.standard_normal((NEXP, 2 * D, DH)) / np.sqrt(2 * D)).astype(np.float32),
        "b1": np.zeros((NEXP, DH), np.float32),
        "W2": (rng.standard_normal((NEXP, DH, D)) / np.sqrt(DH)).astype(np.float32),
        "b2": np.zeros((NEXP, D), np.float32),
        "Wq": (rng.standard_normal((D, DP)) / np.sqrt(D)).astype(np.float32),
        "bq": np.zeros(DP, np.float32),
        "Wk": (rng.standard_normal((D, DP)) / np.sqrt(D)).astype(np.float32),
        "bk": np.zeros(DP, np.float32),
    }
    out = kernel(**inputs)
    print("out", out.shape, out.dtype, float(np.abs(out).mean()))

